# revision 20
# baseline (speedup 1.0000x reference)
"""Trainium2 Bass kernel for nn_CustomConv1D_d (rank-1 dense conv1d, stride 21).

Math: out[b, t, o] = r[b, t] for all o in [0, 237), where
  r[b, t] = sum_k w[k] * sum_c x[b, 21 t + k, c],  w = softmax(p3*i + p4*i^2).

Strategy (pure data parallel over batch, 4 batches per core):
  - Per core, view x as flat [43008, 237]; each output group t owns 21
    consecutive rows = 19908 contiguous bytes. Load tiles [128 groups,
    21*237] — one fully-contiguous 19908B DMA descriptor per partition.
  - Per tile, DVE does a segmented reduce over channels -> [128, 21]
    per-tap sums, multiplies by the tap weights, and reduces over taps
    -> r[128, 1] — already in output layout.
  - ACT engine broadcasts each r column across 237 channels; ACT-issued
    DMAs stream results out without stalling the input DMA ring.
"""

import numpy as np
from contextlib import ExitStack

import concourse.bass as bass
import concourse.tile as tile
import concourse.mybir as mybir
from concourse.bass_utils import run_bass_kernel_spmd

TAPS = 21
C = 237
B = 32
L = 10752
T = 512
NCORES = 8
BPC = B // NCORES            # 4 batches per core
ROWS = BPC * L               # 43008 rows per core
GROUPS = BPC * T             # 2048 groups per core
NQ = GROUPS // 128           # 16 tiles of 128 groups
GROUP_ROWS = 128 * TAPS      # 2688 input rows per tile
FD = TAPS * C                # 4977 elements per group
CP = C + 1                   # channel segment padded to even length (DVE 2x mode)
FDP = TAPS * CP              # padded free dim per group
OBLK = 4                     # group-tiles per output tile
F32 = mybir.dt.float32


class _TileContext(tile.TileContext):
    """TileContext with a post-scheduling pass that splits instructions
    carrying >1 sem wait onto preceding single-wait nops on the same
    engine — the pinned neuronxcc rejects instructions with multiple
    sync wait commands."""

    def schedule_and_allocate(self):
        ret = super().schedule_and_allocate()
        self._split_multi_waits()
        return ret

    def _split_multi_waits(self):
        nc = self.nc
        for fn in nc.m.functions:
            for bb in fn.blocks:
                if not any(
                    inst.sync_info
                    and inst.sync_info.on_wait
                    and len(inst.sync_info.on_wait) > 1
                    for inst in bb.instructions
                ):
                    continue
                new_insts = []
                for inst in bb.instructions:
                    si = inst.sync_info
                    waits = list(si.on_wait) if si and si.on_wait else []
                    if len(waits) > 1:
                        si.on_wait = waits[-1:]
                        for w in waits[:-1]:
                            nop = mybir.InstNoOp(
                                name=f"I-splitw-{nc.next_id()}",
                                engine=inst.engine,
                                sync_info=mybir.SyncInfo(on_wait=[w], on_update=[]),
                            )
                            nc.register_instruction(nop, overwrite=True)
                            new_insts.append(nop)
                    new_insts.append(inst)
                bb.instructions[:] = new_insts


def _build():
    nc = bass.Bass("TRN2", target_bir_lowering=False, debug=False)
    x = nc.dram_tensor("x", [ROWS, C], F32, kind="ExternalInput").ap()
    wv = nc.dram_tensor("wv", [OBLK * TAPS], F32, kind="ExternalInput").ap()
    y = nc.dram_tensor("y", [GROUPS, C], F32, kind="ExternalOutput").ap()

    with _TileContext(nc) as tc:
        with ExitStack() as ctx:
            xin = ctx.enter_context(tc.tile_pool(name="xin", bufs=4))
            kp = ctx.enter_context(tc.tile_pool(name="kp", bufs=3))
            sp = ctx.enter_context(tc.tile_pool(name="sp", bufs=1))
            op = ctx.enter_context(tc.tile_pool(name="op", bufs=2))

            wrep = sp.tile([128, OBLK * TAPS], F32)
            nc.gpsimd.dma_start(wrep[:], wv[None, :].broadcast_to([128, OBLK * TAPS]))
            acc_all = sp.tile([128, NQ], F32)

            # Tap-split schedule per tile: the first tiles land in chunks so
            # DVE starts early (a whole tile takes ~25us to arrive with 4
            # DMAs round-robining the ring); the final tiles stream in
            # chunks so the post-DMA serial tail stays short.
            splits = {
                0: [6, 5, 5, 5],
                1: [11, 10],
                NQ - 2: [11, 10],
                NQ - 1: [7, 6, 6, 2],
            }

            # Each output group: per-tap channel sums per tile, then one
            # batched weight-multiply + tap-reduce for the whole group.
            out_groups = [[0, 1, 2, 3], [4, 5, 6, 7], [8, 9, 10, 11], [12, 13, 14], [15]]
            for qs in out_groups:
                nb = len(qs)
                skg = kp.tile([128, OBLK * TAPS], F32, tag="skg")
                for j, q in enumerate(qs):
                    xt = xin.tile([128, FD], F32, tag="xt")
                    v3 = xt.rearrange("p (k c) -> p k c", c=C)
                    src = x[q * GROUP_ROWS : (q + 1) * GROUP_ROWS, :].rearrange(
                        "(p k) c -> p k c", k=TAPS
                    )
                    k0 = 0
                    for tk in splits.get(q, [TAPS]):
                        nc.sync.dma_start(
                            v3[:, k0 : k0 + tk, :],
                            src[:, k0 : k0 + tk, :],
                        )
                        nc.vector.reduce_sum(
                            skg[:, j * TAPS + k0 : j * TAPS + k0 + tk],
                            v3[:, k0 : k0 + tk, :],
                            axis=mybir.AxisListType.X,
                        )
                        k0 += tk
                skw = kp.tile([128, OBLK * TAPS], F32, tag="skw")
                nc.vector.tensor_mul(
                    skw[:, 0 : nb * TAPS], skg[:, 0 : nb * TAPS], wrep[:, 0 : nb * TAPS]
                )
                nc.vector.reduce_sum(
                    acc_all[:, qs[0] : qs[0] + nb],
                    skw[:, 0 : nb * TAPS].rearrange("p (o k) -> p o k", k=TAPS),
                    axis=mybir.AxisListType.X,
                )

                osb = op.tile([128, OBLK * C], F32, tag="osb")
                for j, qg in enumerate(qs):
                    nc.scalar.activation(
                        osb[:, j * C : (j + 1) * C],
                        acc_all[:, qg : qg + 1].broadcast_to([128, C]),
                        mybir.ActivationFunctionType.Identity,
                    )
                nc.scalar.dma_start(
                    y[qs[0] * 128 : (qs[-1] + 1) * 128, :].rearrange(
                        "(q p) c -> p q c", p=128
                    ),
                    osb[:, 0 : nb * C].rearrange("p (q c) -> p q c", c=C),
                )
    return nc


_NC_CACHE = {}


def _get_nc():
    if "nc" not in _NC_CACHE:
        _NC_CACHE["nc"] = _build()
    return _NC_CACHE["nc"]


def _tap_weights(param3: float, param4: float) -> np.ndarray:
    i = np.arange(1, TAPS + 1, dtype=np.float32)
    logits = (np.float32(param3) * i + np.float32(param4) * i * i).astype(np.float32)
    e = np.exp(logits - logits.max(), dtype=np.float32)
    w = (e / e.sum()).astype(np.float32)
    return np.tile(w, OBLK).astype(np.float32)  # [OBLK*TAPS]


def run_with_results(inputs, **spmd_kwargs):
    x = np.ascontiguousarray(np.asarray(inputs["inputs"], dtype=np.float32))
    assert x.shape == (B, L, C), x.shape
    wv = _tap_weights(
        float(np.asarray(inputs["param3"])), float(np.asarray(inputs["param4"]))
    )
    xs = x.reshape(NCORES, ROWS, C)
    in_maps = [{"x": xs[i], "wv": wv} for i in range(NCORES)]
    res = run_bass_kernel_spmd(_get_nc(), in_maps, list(range(NCORES)), **spmd_kwargs)
    out = np.stack([res.results[i]["y"] for i in range(NCORES)])
    return out.reshape(B, T, C).astype(np.float32, copy=False), res


def kernel(**inputs) -> np.ndarray:
    out, _ = run_with_results(inputs)
    return out
